# revision 44
# baseline (speedup 1.0000x reference)
"""Trainium2 Bass kernel for nn_Net_18906446037087 (snntorch Leaky SNN layer).

Reference semantics (per batch element, 255 steps, f32):
    cur = x @ W.T                         # [B, 1]
    m_0 = 0
    m_{t+1} = (0.95*m_t + cur) * (m_t <= 1)
    spk_{t+1} = (m_{t+1} > 1)
Outputs: (spk_rec, mem_rec), each [255, B, 1] f32.

Sharding: pure data parallel over batch across 8 cores (B=65536 -> 8192/core).

Closed form: the trajectory is periodic in t. With s[k] = (1-b^k)/(1-b),
an element first spikes at step K iff cur > 1/s[K]; then mem repeats the
pattern A_K[t] = s[((t-1) mod (K+1)) + 1] (0 at the reset slot); elements
with cur <= 1/s[255] follow the pure ramp R[t] = s[t]. So
    mem[t, b] = cur_b * (R[t] + sum_{k>=K(b)} (A_k - A_{k+1})[t])
which is one matmul  mem = G^T @ F  with
    G[0] = R, G[k] = A_k - A_{k+1} (A_256 := R)     (host-precomputed)
    F[k, b] = cur_b * [cur_b > theta_k], theta_0 = -inf, theta_k = 1/s[k].
spk is derived on host as mem > 1.0 (exact).

Numerics: the matvec and the G@F matmul run in fp32r (PE rounds operands
to 11 explicit mantissa bits, RNE — measured on HW); end-to-end rel err
~2e-3 vs the 2e-2 gate (validated against the axon-backend oracle).

Per-core pipeline (B_CORE=8192 = 16 groups of 512 = 64 subgroups of 128):
  per group g:
   - two feature-split DMAs load x rows as [128, 4, 784] so the first
     transpose pairs start after the first half-load;
   - PE transposes the 7 K-chunks (fp32r, 1.5 cyc/row) in pairs into
     [128,1024] two-bank PSUM tiles; DVE/ACT evacuate with one copy per
     pair; 28 matmuls with xT *stationary* and W moving (out
     [128,1]/subgroup; the PSUM bank is pre-zeroed and all matmuls use
     start=False because a start=True matmul clobbers the other open
     accumulation regions in its bank) -> cur columns;
   - four single-column PE transposes form a [1,512] cur row in one PSUM
     bank (first start=True, rest accumulate), DVE copies it out as
     fp32r;
   - PE broadcasts the row to [128,512] via a ones outer product (fp32r),
     ACT evacuates, DVE stts build F (2 class chunks), two accumulated
     fp32r matmuls per 128-step slab produce mem[t,b] in [t-partition,
     batch-free] layout, DVE/ACT evacuate, ACT issues contiguous
     2KB-per-partition writes (>=512B runs, no DMA latency penalty).
Scheduling: two-deep software pipeline — in iteration g PE emits G@F for
group g-2 and the broadcast for g-1 BEFORE the x-dependent transposes of
g, so no tail stage queues behind a stalled x wait. DMA issue queues:
x loads alone on SP (nothing can block the x stream); tail DMAs on ACT
right after their producing copies; all constants packed into a single
[128,775] tensor loaded with one DMA before the first x tile.

Output DMAs for groups <= 12 are buffered in SBUF (opool bufs=28) and
flushed from SP after the last x load: x streams back-to-back on the DMA
engines (ends ~74us) and the buffered outputs drain while the final
groups' tails compute.

TimelineSim: 103375 ns/core (baseline 260328, 2.52x; DMA floor ~96us
busy: x in 71.4 + mem out 23.2 at 360GB/s aggregate).
"""
import sys
if "/opt/trn_rl_repo" not in sys.path:
    sys.path.insert(0, "/opt/trn_rl_repo")

import numpy as np
from contextlib import ExitStack

import concourse.bass as bass
import concourse.bacc as bacc
import concourse.mybir as mybir
import concourse.tile as tile
from concourse.bass_utils import run_bass_kernel_spmd

F32 = mybir.dt.float32
F32R = mybir.dt.float32r
ALU = mybir.AluOpType

N_CORES = 8
B_FULL = 65536
B_CORE = B_FULL // N_CORES          # 8192
D = 784
NUM_STEPS = 255
BETA = 0.95
THRESHOLD = 1.0

GROUP = 512                          # batch per group
NGROUP = B_CORE // GROUP             # 16
CHUNKS = [(0, 128), (128, 128), (256, 128), (384, 128), (512, 128), (640, 128), (768, 16)]
NCLASS = 256                         # class 0 = ramp; class k = first spike at k
TCHUNKS = [(0, 128), (128, 127)]     # step slabs (255 rows)


def _build():
    nc = bacc.Bacc("TRN2", target_bir_lowering=False, debug=False,
                   num_devices=N_CORES)
    x_d = nc.dram_tensor("x", [B_CORE, D], F32R, kind="ExternalInput")
    # all constants packed in one tensor/DMA: [w 0:7 | ident 7:135 |
    # gtab 135:645 | thr 645:647 | ones row 647:775]
    c_d = nc.dram_tensor("consts", [128, 775], F32R, kind="ExternalInput")
    mem_d = nc.dram_tensor("mem", [NUM_STEPS, B_CORE], F32, kind="ExternalOutput")

    with tile.TileContext(nc) as tc, ExitStack() as ctx:
        xpool = ctx.enter_context(tc.tile_pool(name="xpool", bufs=5))
        xtpool = ctx.enter_context(tc.tile_pool(name="xtpool", bufs=2))
        rowpool = ctx.enter_context(tc.tile_pool(name="rowpool", bufs=2))
        fpool = ctx.enter_context(tc.tile_pool(name="fpool", bufs=2))
        opool = ctx.enter_context(tc.tile_pool(name="opool", bufs=28))
        const = ctx.enter_context(tc.tile_pool(name="const", bufs=1))
        psxt = ctx.enter_context(tc.tile_pool(name="psxt", bufs=1, space="PSUM"))
        psacc = ctx.enter_context(tc.tile_pool(name="psacc", bufs=1, space="PSUM"))
        psbc = ctx.enter_context(tc.tile_pool(name="psbc", bufs=1, space="PSUM"))
        psgo = ctx.enter_context(tc.tile_pool(name="psgo", bufs=2, space="PSUM"))

        c_all = const.tile([128, 775], F32R, name="c_all")
        w_t = c_all[:, 0:7].bitcast(F32)
        id_t = c_all[:, 7:135]
        g_t = c_all[:, 135:645]
        thr_t = c_all[:, 645:647].bitcast(F32)
        ones_t = c_all[0:1, 647:775]

        cur_cols = const.tile([128, NGROUP * 4], F32, name="cur_cols")

        copy_engines = [nc.vector.tensor_copy, nc.scalar.copy]
        copy_idx = [0]

        def copy(out, in_):
            eng = copy_engines[copy_idx[0] % len(copy_engines)]
            copy_idx[0] += 1
            eng(out, in_)

        # x rows for group g as [128, 4, D]: partition p, subgroup j, feature
        x_grp = x_d[:].rearrange("(g j p) f -> g p j f", g=NGROUP, j=4)

        def load_x(g):
            # split by feature so the first transpose pairs (chunks 0-2)
            # start after the first half-load
            xg4 = xpool.tile([128, 4, D], F32R, tag="xg", name="xg4")
            nc.sync.dma_start(xg4[:, :, 0:384], x_grp[g][:, :, 0:384])
            nc.sync.dma_start(xg4[:, :, 384:D], x_grp[g][:, :, 384:D])
            return xg4

        PAIRS = [(0, 1), (2,), (3, 4), (5, 6)]

        def group_transposes(g, xg4):
            """Transpose group g's 7 K-chunks into SBUF pair tiles.

            Chunks go in pairs into [128,1024] two-bank PSUM tiles and are
            evacuated with one copy per pair (fewer queue slots/sems).
            """
            xts = {}
            for pi, pair in enumerate(PAIRS):
                xt_ps = psxt.tile([128, 2 * GROUP], F32R,
                                  tag=f"xt{pi % 2}", name="xt_ps")
                w_cols = 0
                for k, ci in enumerate(pair):
                    c0, cl = CHUNKS[ci]
                    for j in range(4):
                        nc.tensor.transpose(
                            xt_ps[:cl, k * GROUP + j * 128:k * GROUP + (j + 1) * 128],
                            xg4[:, j, c0:c0 + cl],
                            id_t,
                        )
                    w_cols = (k + 1) * GROUP
                xt_sb = xtpool.tile([128, 2 * GROUP], F32R,
                                    tag=f"xtsb{pi % 2}", name="xt_sb")
                copy(xt_sb[:, :w_cols], xt_ps[:, :w_cols])
                for k, ci in enumerate(pair):
                    xts[ci] = xt_sb[:, k * GROUP:(k + 1) * GROUP]
            return xts

        def group_accum(g, xts):
            """28 xT-stationary matmuls -> cur columns for group g."""
            acc = psacc.tile([128, 4], F32, tag="acc")
            nc.vector.memset(acc[:, :], 0.0)
            for ci, (c0, cl) in enumerate(CHUNKS):
                for j in range(4):
                    nc.tensor.matmul(
                        acc[:, j:j + 1],
                        xts[ci][:cl, j * 128:(j + 1) * 128].bitcast(F32),
                        w_t[:cl, ci:ci + 1],
                        start=False,
                        stop=(ci == len(CHUNKS) - 1),
                    )
            nc.vector.tensor_copy(cur_cols[:, 4 * g:4 * g + 4], acc[:, :])

        def group_rowform(g):
            """Transpose group g's cur columns into a [1,512] SBUF row.

            Four single-column PE transposes target disjoint 128-wide spans
            of one PSUM bank; the first uses start=True (resets the bank),
            the rest accumulate, avoiding the whole-bank reset clobber.
            """
            row_ps = psbc.tile([1, GROUP], F32, tag="bc", name="row_ps")
            for c in range(4):
                nc.tensor.matmul(
                    row_ps[0:1, c * 128:(c + 1) * 128],
                    cur_cols[:, 4 * g + c:4 * g + c + 1],
                    id_t.bitcast(F32),
                    start=(c == 0), stop=(c == 3), is_transpose=True)
            cur_row = rowpool.tile([1, GROUP], F32R, tag="row")
            nc.vector.tensor_copy(cur_row[:, :], row_ps[:, :])
            return cur_row

        def tail_bcast(g, cur_row):
            """Broadcast group g's cur row to [128,512] (PE + ACT copy)."""
            bc_ps = psbc.tile([128, GROUP], F32, tag="bc")
            nc.tensor.matmul(bc_ps[:, :], ones_t, cur_row[0:1, :],
                             start=True, stop=True)
            bc_sb = fpool.tile([128, GROUP], F32, tag="bc_sb")
            nc.scalar.copy(bc_sb[:, :], bc_ps[:, :])
            return bc_sb

        def tail_stts(g, bc_sb):
            """Build F for both class chunks (DVE)."""
            fts = []
            for c in range(2):
                ft = fpool.tile([128, GROUP], F32R, tag=f"f{c}")
                nc.vector.scalar_tensor_tensor(
                    ft[:, :], bc_sb[:, :], thr_t[:, c:c + 1], bc_sb[:, :],
                    ALU.is_gt, ALU.mult)
                fts.append(ft)
            return fts

        deferred_outs = []

        def tail_back(g, fts):
            """G@F matmuls and evacuation for group g.

            Output DMAs for early groups are deferred and issued from SP
            after the last x load: x then streams back-to-back on the DMA
            engines (ending ~73us) and the buffered outputs flush while
            the final tails compute, instead of stretching the x stream.
            """
            osbs = []
            for tc_i, (t0, tl) in enumerate(TCHUNKS):
                go_ps = psgo.tile([128, GROUP], F32, tag="go")
                for c in range(2):
                    nc.tensor.matmul(
                        go_ps[:tl, :],
                        g_t[:, c * NUM_STEPS + t0:c * NUM_STEPS + t0 + tl],
                        fts[c][:, :],
                        start=(c == 0), stop=(c == 1))
                o_sb = opool.tile([128, GROUP], F32, tag="osb")
                if tc_i == 0:
                    nc.vector.tensor_copy(o_sb[:tl, :], go_ps[:tl, :])
                else:
                    nc.scalar.copy(o_sb[:tl, :], go_ps[:tl, :])
                osbs.append((t0, tl, o_sb))
            if g <= NGROUP - 4:
                for t0, tl, o_sb in osbs:
                    deferred_outs.append((g, t0, tl, o_sb))
            else:
                for t0, tl, o_sb in reversed(osbs):
                    nc.scalar.dma_start(
                        mem_d[t0:t0 + tl, g * GROUP:(g + 1) * GROUP],
                        o_sb[:tl, :])

        nc.sync.dma_start(c_all[:], c_d[:])
        # Two-deep software pipeline: in iteration g the PE emits, in
        # order, G@F for group g-2 (F ready since last iteration), the cur
        # broadcast for g-1 (row copied last iteration), then the
        # x-dependent transposes of g — so no PE stage ever queues behind
        # a stalled x wait. The F stts for g-1 go after the pair copies so
        # DVE's queue head never waits on the ACT bc copy.
        prev1 = prev2 = None
        for g in range(NGROUP):
            xg4 = load_x(g)
            if g == NGROUP - 1:
                # flush deferred outputs from SP: x loads are all issued,
                # so these stream back-to-back behind them
                for dg, t0, tl, o_sb in deferred_outs:
                    nc.sync.dma_start(
                        mem_d[t0:t0 + tl, dg * GROUP:(dg + 1) * GROUP],
                        o_sb[:tl, :])
                deferred_outs.clear()
            if prev2 is not None:
                tail_back(*prev2)
            if prev1 is not None:
                p1g, prow = prev1
                pbc = tail_bcast(p1g, prow)
            xts = group_transposes(g, xg4)
            prev2 = (p1g, tail_stts(p1g, pbc)) if prev1 is not None else None
            group_accum(g, xts)
            prev1 = (g, group_rowform(g))
        tail_back(*prev2)
        p1g, prow = prev1
        tail_back(p1g, tail_stts(p1g, tail_bcast(p1g, prow)))

    nc.compile()
    return nc


_NC_CACHE = None


def _get_nc():
    global _NC_CACHE
    if _NC_CACHE is None:
        _NC_CACHE = _build()
    return _NC_CACHE


def _round11(a):
    """Round-to-nearest-even at 11 explicit mantissa bits (fp32r grid)."""
    u = np.ascontiguousarray(a, np.float32).view(np.uint32)
    u = (u + 0x800) & 0xFFFFF000
    return u.view(np.float32)


def _host_tables():
    s = np.zeros(NUM_STEPS + 2)
    for k in range(1, NUM_STEPS + 2):
        s[k] = s[k - 1] * BETA + 1.0
    t = np.arange(1, NUM_STEPS + 1)
    R = s[t]

    def pattern(k):
        P = k + 1
        phi = ((t - 1) % P) + 1
        v = s[phi].copy()
        v[phi == P] = 0.0
        return v

    G = np.zeros((NCLASS, NUM_STEPS))
    G[0] = R
    for k in range(1, NCLASS):
        Ak = pattern(k)
        Ak1 = pattern(k + 1) if k + 1 < NCLASS else R
        G[k] = Ak - Ak1
    # gtab layout: [128 partitions, 2 chunks * 255] , class = c*128 + p
    gtab = np.zeros((128, 2 * NUM_STEPS), np.float32)
    for c in range(2):
        gtab[:, c * NUM_STEPS:(c + 1) * NUM_STEPS] = G[c * 128:(c + 1) * 128]
    gtab = _round11(gtab)

    thr = np.zeros((128, 2), np.float32)
    theta = (1.0 / s[1:NCLASS]).astype(np.float32)  # theta_k, k=1..255
    flat = np.concatenate([[np.float32(-3.0e38)], theta])
    thr[:, 0] = flat[0:128]
    thr[:, 1] = flat[128:256]
    return gtab, thr


def _prep_inputs(x, W):
    x = np.ascontiguousarray(np.asarray(x, dtype=np.float32))
    W = np.asarray(W, dtype=np.float32).reshape(-1)
    assert x.shape == (B_FULL, D) and W.shape == (D,)
    wpad = np.zeros(896, np.float32)
    wpad[:D] = W
    wcol = np.ascontiguousarray(wpad.reshape(7, 128).T)
    gtab, thr = _host_tables()
    consts = np.zeros((128, 775), np.float32)
    consts[:, 0:7] = wcol
    consts[:, 7:135] = np.eye(128, dtype=np.float32)
    consts[:, 135:645] = gtab
    consts[:, 645:647] = thr
    consts[0, 647:775] = 1.0
    in_maps = [
        {"x": x[d * B_CORE:(d + 1) * B_CORE], "consts": consts}
        for d in range(N_CORES)
    ]
    return in_maps


def kernel(x, W, _trace=False, _trace_kwargs=None):
    nc = _get_nc()
    in_maps = _prep_inputs(x, W)
    res = run_bass_kernel_spmd(nc, in_maps, list(range(N_CORES)),
                               trace=_trace, **(_trace_kwargs or {}))
    mem = np.concatenate([res.results[d]["mem"] for d in range(N_CORES)], axis=1)
    mem_rec = mem.reshape(NUM_STEPS, B_FULL, 1)
    spk_rec = (mem_rec > np.float32(THRESHOLD)).astype(np.float32)
    if _trace:
        return (spk_rec, mem_rec), res
    return spk_rec, mem_rec
